# revision 7
# baseline (speedup 1.0000x reference)
"""Bass/Trainium2 kernel for nn_Conv2d_mvm (bit-sliced analog-crossbar conv2d).

The reference's bit-slice / bit-stream decomposition is mathematically lossless:
  - weight slices recombine exactly to wq = round(w * 256)            (int)
  - input bit-streams recombine exactly to patches = im2col(round(x*256))
so the whole model is exactly:
    out_int = conv2d(xq, wq, pad=1)               (int32, exact)
    out     = clip(out_int >> 4, -2^15, 2^15-1) / 4096 + bias

Ranges (verified): |xq| <= ~1224, |wq| <= ~89, |out_int| < 2^22.
Therefore fp16 operands with fp32 PSUM accumulation compute out_int exactly.

Sharding: data-parallel over batch, 1 image per NeuronCore (8 cores).

Per-core device pipeline:
  1. ONE input DMA: packed fp32 buffer [64, 1733] holding padded x
     (rows 0-31, cols 0-1155), weights lhsT (rows 0-31, cols 1156-1731,
     as fp32) and bias (rows 0-63, col 1732).  A single DMA keeps the
     number of DMA-queue semaphores low (the Tile end-of-kernel drain has
     a small HW wait-slot budget).
  2. Quantize on device: xq = round_half_even(x*256) via the 1.5*2^23
     magic-number trick (exact RNE, matches np.round), output fp16;
     weights converted fp32 -> fp16 (exact, small ints).
  3. 2 spatial halves x 9 taps accumulating matmuls into PSUM [64,512]
     (contract = Cin=32; rhs is a strided 3D view of the padded image).
  4. Postprocess: fp32->int32 convert (exact: values are integers),
     arithmetic shift right 4, clip, scale 1/4096, +bias -> fp32.
  5. ONE output DMA [64, 1024] -> host reshapes to [8,64,32,32].
"""

import numpy as np

import concourse.bass as bass
import concourse.mybir as mybir
import concourse.tile as tile
from concourse.bass_utils import run_bass_kernel_spmd

N_CORES = 8
MAGIC = 12582912.0  # 1.5 * 2**23: float add forces round-to-nearest-even int


class SplitDrainTileContext(tile.TileContext):
    """TileContext whose end-of-kernel drain splits its semaphore waits
    across multiple single-wait Drain instructions.

    The stock TileContext attaches one wait per live proc (engine/DMA-lane
    semaphore) to a single SP Drain; TRN2 instructions only encode one
    sync-wait command, so walrus rejects the program ("Too many sync wait
    commands") whenever >1 procs are in flight at kernel end.
    """

    def _drain_and_barrier(self, tick_clock, wait_clock):
        from concourse.vector_clock import ScopedClock

        nc = self.nc
        drain_inst = nc.sync.drain()
        wait_clock.add_sem_waits(
            drain_inst.ins, ScopedClock({None: tick_clock.global_clock})
        )
        waits = list(drain_inst.ins.sync_info.on_wait)
        if len(waits) > 1:
            drain_inst.ins.sync_info = mybir.SyncInfo(
                on_wait=[waits[0]],
                on_update=list(drain_inst.ins.sync_info.on_update),
            )
            for w in waits[1:]:
                extra = nc.sync.drain()
                extra.ins.sync_info = mybir.SyncInfo(on_wait=[w], on_update=[])

        nc.all_engine_barrier()
        assert self.sems is not None
        popped = nc._tile_sem_poison_stack.pop()
        assert popped is self._sem_poison
        nc.clear_and_free_semaphores(list(self.sems.allocated().values()))
        nc.all_engine_barrier()
CIN, COUT, H, W = 32, 64, 32, 32
PH, PW = H + 2, W + 2  # padded
NPIX = H * W  # 1024
XCOLS = PH * PW          # 1156
WCOLS = 9 * COUT         # 576
BCOL = XCOLS + WCOLS     # 1732
INCOLS = BCOL + 1        # 1733

_CACHE = {}


def _build_module():
    nc = bass.Bass("TRN2", target_bir_lowering=False, debug=False)

    in_d = nc.dram_tensor("packed", [COUT, INCOLS], mybir.dt.float32,
                          kind="ExternalInput")
    y_d = nc.dram_tensor("y", [COUT, NPIX], mybir.dt.float32,
                         kind="ExternalOutput")

    AL = mybir.AluOpType

    with SplitDrainTileContext(nc) as tc:
        from contextlib import ExitStack
        with ExitStack() as ctx:
            io = ctx.enter_context(tc.tile_pool(name="io", bufs=1))
            work = ctx.enter_context(tc.tile_pool(name="work", bufs=2))
            pp = ctx.enter_context(tc.tile_pool(name="psum", bufs=2, space="PSUM"))

            pk = io.tile([COUT, INCOLS], mybir.dt.float32, tag="pk")
            nc.sync.dma_start(out=pk[:], in_=in_d[:])
            x_ap = pk[0:CIN, 0:XCOLS]
            w_ap = pk[0:CIN, XCOLS:BCOL]
            b_ap = pk[0:COUT, BCOL:INCOLS]

            # weights fp32 -> fp16 (exact: small integers)
            wt = io.tile([CIN, WCOLS], mybir.dt.float16, tag="wt")
            nc.vector.tensor_copy(wt[:], w_ap)

            # quantize: xq = RNE(x*256), as fp16 (exact: |xq| < 2048)
            q1 = io.tile([CIN, XCOLS], mybir.dt.float32, tag="q1")
            nc.vector.tensor_scalar(out=q1[:], in0=x_ap, scalar1=256.0,
                                    scalar2=MAGIC, op0=AL.mult, op1=AL.add)
            xq = io.tile([CIN, XCOLS], mybir.dt.float16, tag="xq")
            nc.vector.tensor_scalar(out=xq[:], in0=q1[:], scalar1=-MAGIC,
                                    scalar2=None, op0=AL.add)
            xq3 = xq[:].rearrange("p (r c) -> p r c", c=PW)

            o = io.tile([COUT, NPIX], mybir.dt.float32, tag="o")
            for h in range(2):  # spatial halves: output rows [16h, 16h+16)
                ps = pp.tile([COUT, 512], mybir.dt.float32, tag="ps")
                for t in range(9):
                    di, dj = t // 3, t % 3
                    rhs = xq3[:, di + 16 * h: di + 16 * h + 16, dj: dj + W]
                    nc.tensor.matmul(ps[:], wt[:, t * COUT:(t + 1) * COUT], rhs,
                                     start=(t == 0), stop=(t == 8))
                # psum fp32 (exact integers) -> int32
                i32 = work.tile([COUT, 512], mybir.dt.int32, tag="i32")
                nc.vector.tensor_copy(i32[:], ps[:])
                s1 = work.tile([COUT, 512], mybir.dt.int32, tag="s1")
                nc.vector.tensor_scalar(out=s1[:], in0=i32[:], scalar1=4,
                                        scalar2=None, op0=AL.arith_shift_right)
                s2 = work.tile([COUT, 512], mybir.dt.float32, tag="s2")
                nc.vector.tensor_scalar(out=s2[:], in0=s1[:], scalar1=32767,
                                        scalar2=-32768, op0=AL.min, op1=AL.max)
                nc.vector.tensor_scalar(out=o[:, 512 * h: 512 * (h + 1)],
                                        in0=s2[:], scalar1=1.0 / 4096.0,
                                        scalar2=b_ap, op0=AL.mult, op1=AL.add)
            nc.sync.dma_start(out=y_d[:], in_=o[:])

    return nc


def get_nc():
    if "nc" not in _CACHE:
        _CACHE["nc"] = _build_module()
    return _CACHE["nc"]


def prep_in_maps(x, weight, bias):
    x = np.asarray(x, dtype=np.float32)
    weight = np.asarray(weight, dtype=np.float32)
    bias = np.asarray(bias, dtype=np.float32)

    # weight quantization (host): wq = round_half_even(w*256); |wq| <= ~89
    wq = np.round(weight * np.float32(256.0)).astype(np.float32)
    # lhsT[ci, (di*3+dj)*64 + co] = wq[co, ci, di, dj]
    lhsT = wq.transpose(1, 2, 3, 0).reshape(CIN, WCOLS)

    in_maps = []
    for c in range(N_CORES):
        buf = np.zeros((COUT, INCOLS), dtype=np.float32)
        xpad = np.pad(x[c], ((0, 0), (1, 1), (1, 1)))
        buf[0:CIN, 0:XCOLS] = xpad.reshape(CIN, XCOLS)
        buf[0:CIN, XCOLS:BCOL] = lhsT
        buf[0:COUT, BCOL] = bias
        in_maps.append({"packed": buf})
    return in_maps


def run_spmd(in_maps, **kw):
    return run_bass_kernel_spmd(get_nc(), in_maps, list(range(N_CORES)), **kw)


def kernel(x, weight, bias):
    res = run_spmd(prep_in_maps(x, weight, bias))
    out = np.stack([r["y"].reshape(COUT, H, W) for r in res.results])
    return out.astype(np.float32)
